# revision 32
# baseline (speedup 1.0000x reference)
"""Chamfer distance kernel for 8 Trainium2 NeuronCores — v7 (shared matrix).

Each core computes ONE [4096 x 8192] block of the per-batch distance
matrix (p1-half rows x all p2 cols) instead of two directional passes:
half the matmul + PSUM-drain volume of v6.  Both reductions come from
the same block (negated, so min becomes max):
  drain: ScalarE ACT Copy with scale=-1 drains PSUM->SBUF bf16 (e = -d);
       ScalarE is the only non-Vector engine that can touch PSUM.
  dir1 (per-p1-row min): VectorE elementwise max-folds across the 4
       groups of a qtile (all share the same 128 queries), then a short
       halving tree to 1024 wide; one batched tail reduces all qtiles.
  dir2 (per-p2-col min): VectorE running elementwise-max chain over
       qtiles into Racc [128, 8192]; host max-combines the two cores of
       each batch and reduces over the partition axis (the
       "min-combining" step of the sharding hint).
All VectorE reductions are bf16 tensor_tensor ops, the only reduce
path that reaches the DVE 2x perf mode (tensor_reduce/pool are 1x).
"""

import numpy as np
import ml_dtypes

bf16 = ml_dtypes.bfloat16

B = 4
N = 8192            # points per cloud
NQ = N // 2         # p1 rows handled per core
K = 13              # real contraction rows (padded to 32 per band)
KP = 32
QT = NQ // 128      # query tiles per core (32)
NCHUNK = 512        # db points per matmul (one PSUM bank stripe)
GROUP = 4           # chunks per PSUM group tile
GSZ = GROUP * NCHUNK            # 2048
NGROUP = N // GSZ   # 4 groups per qtile
N_CORES = 8
BIGF = 1.0e30
TREE_OUT = 1024     # per-qtile dir1 tree stops here; batched tail finishes


def build_bass():
    import concourse.bacc as bacc
    import concourse.mybir as mybir
    from concourse.tile import TileContext

    fp32 = mybir.dt.float32
    bfl6 = mybir.dt.bfloat16
    A = mybir.AluOpType
    AX = mybir.AxisListType
    ACTF = mybir.ActivationFunctionType

    nc = bacc.Bacc()

    la = nc.declare_dram_parameter("la", [128, NQ], bfl6, isOutput=False)
    ra = nc.declare_dram_parameter("ra", [128, N], bfl6, isOutput=False)
    out1 = nc.declare_dram_parameter("out1", [128, 1], fp32, isOutput=True)
    out2 = nc.declare_dram_parameter("out2", [128, N], bfl6, isOutput=True)

    with TileContext(nc) as tc:
        with (
            tc.tile_pool(name="ops", bufs=1) as ops_pool,
            tc.tile_pool(name="psum", bufs=2, space="PSUM") as psum_pool,
            tc.tile_pool(name="exit", bufs=4) as e_pool,
            tc.tile_pool(name="scrap", bufs=2) as sc_pool,
        ):
            L0 = ops_pool.tile([128, 128], bfl6, tag="L0", name="L0")
            L1 = ops_pool.tile([128, NQ - 128], bfl6, tag="L1", name="L1")
            # group 0 split per matmul band so the very first matmul only
            # waits on a 128 KB DMA sliver
            Rf0 = [ops_pool.tile([128, NCHUNK], bfl6, tag=f"Rf0{b}",
                                 name=f"Rf0{b}")
                   for b in range(GROUP)]
            RfT = [None] + [ops_pool.tile([128, GSZ], bfl6, tag=f"Rf{g}",
                                          name=f"Rf{g}")
                            for g in range(1, NGROUP)]
            Racc = ops_pool.tile([128, N], bfl6, tag="Racc", name="Racc")
            W = ops_pool.tile([128, QT * TREE_OUT], bfl6, tag="W")
            qmin = ops_pool.tile([128, QT], fp32, tag="qmin")
            accsum = ops_pool.tile([128, 1], fp32, tag="accsum")

            # separate tiles so the first matmuls depend only on the
            # first slivers of input (tile-granular DMA dependencies)
            nc.scalar.dma_start(out=L0[:, :], in_=la[:, 0:128])
            for b in range(GROUP):
                nc.sync.dma_start(out=Rf0[b][:, :],
                                  in_=ra[:, b * NCHUNK:(b + 1) * NCHUNK])
            # L1 on the gpsimd SWDGE queue: keeps the ACT queue free for
            # the first drains and the Sync queue free for Rf chunks
            nc.gpsimd.dma_start(out=L1[:, :], in_=la[:, 128:NQ])
            for c in range(1, NGROUP):
                nc.sync.dma_start(out=RfT[c][:, :],
                                  in_=ra[:, c * GSZ:(c + 1) * GSZ])
            for bp in (0, 32, 64, 96):
                nc.tensor.ldweights(L0[bp:bp + KP, 0:128],
                                    tile_position=(bp, 0))
                nc.tensor.ldweights(Rf0[0][bp:bp + KP, 0:128],
                                    tile_position=(bp, 0))

            for t in range(QT):
                last = t == QT - 1
                M = sc_pool.tile([128, GSZ], bfl6, tag="m")
                e4 = e_pool.tile([128, N], bfl6, tag="e4")
                for g in range(NGROUP):
                    pg = psum_pool.tile([128, GSZ], fp32, tag="pg")
                    for band in range(GROUP):
                        bp = 32 * band
                        lhsT = (L0[bp:bp + KP, 0:128] if t == 0 else
                                L1[bp:bp + KP, (t - 1) * 128:t * 128])
                        rhs = (Rf0[band][bp:bp + KP, 0:NCHUNK] if g == 0 else
                               RfT[g][bp:bp + KP,
                                      band * NCHUNK:(band + 1) * NCHUNK])
                        nc.tensor.matmul(
                            pg[:, band * NCHUNK:(band + 1) * NCHUNK],
                            lhsT, rhs,
                            start=True, stop=True,
                            tile_position=(bp, 0),
                        )
                    # e = -d in bf16 (quarter slice of the qtile-wide tile)
                    e = e4[:, g * GSZ:(g + 1) * GSZ]
                    nc.scalar.activation(e, pg[:, :], ACTF.Copy, scale=-1.0)
                    # dir1: elementwise max-fold across groups (same queries)
                    if g == 1:
                        nc.vector.tensor_tensor(
                            out=M[:, :], in0=e4[:, 0:GSZ], in1=e, op=A.max)
                    elif g > 1 and not (last and g == NGROUP - 1):
                        nc.vector.tensor_tensor(
                            out=M[:, :], in0=e, in1=M[:, :], op=A.max)
                # dir2: running col-max chain, full qtile width
                if t == 0:
                    nc.vector.tensor_scalar(
                        out=Racc[:, :], in0=e4[:, :], scalar1=-BIGF,
                        scalar2=None, op0=A.max)
                else:
                    nc.vector.tensor_tensor(
                        out=Racc[:, :], in0=e4[:, :], in1=Racc[:, :],
                        op=A.max)
                if last:
                    # Racc final: ship dir2 now on the idle ACT DMA queue,
                    # then finish the deferred dir1 fold
                    for c in range(NGROUP):
                        nc.scalar.dma_start(
                            out=out2[:, c * GSZ:(c + 1) * GSZ],
                            in_=Racc[:, c * GSZ:(c + 1) * GSZ])
                    nc.vector.tensor_tensor(
                        out=M[:, :], in0=e4[:, (NGROUP - 1) * GSZ:N],
                        in1=M[:, :], op=A.max)
                # dir1 tree: M [2048] -> W[:, t*1024:(t+1)*1024]
                w = GSZ // 2
                while w >= TREE_OUT:
                    dst = (W[:, t * TREE_OUT:(t + 1) * TREE_OUT]
                           if w == TREE_OUT else M[:, 0:w])
                    nc.vector.tensor_tensor(
                        out=dst, in0=M[:, 0:w], in1=M[:, w:2 * w], op=A.max)
                    w //= 2

            # batched tail: [128, QT, TREE_OUT] -> [128, QT]
            Wv = W.rearrange("p (t n) -> p t n", t=QT)
            w = TREE_OUT // 2
            while w >= 1:
                nc.vector.tensor_tensor(
                    out=Wv[:, :, 0:w], in0=Wv[:, :, 0:w],
                    in1=Wv[:, :, w:2 * w], op=A.max)
                w //= 2
            # W holds max(-d) per qtile; dist1 per query = max(-m, 0)
            nc.vector.tensor_scalar(
                out=qmin[:, :], in0=Wv[:, :, 0], scalar1=-1.0, scalar2=0.0,
                op0=A.mult, op1=A.max)
            nc.vector.tensor_reduce(out=accsum[:, :], in_=qmin[:, :],
                                    axis=AX.X, op=A.add)
            nc.sync.dma_start(out=out1[:, :], in_=accsum[:, :])
    nc.finalize()
    return nc


def _split_bf16(x):
    hi = x.astype(bf16)
    lo = (x - hi.astype(np.float32)).astype(bf16)
    return hi, lo


def _pad_bands(rows):
    """[13, n] bf16 -> [128, n]: pad K to 32 with zeros, replicate 4x."""
    n = rows.shape[1]
    k32 = np.zeros((KP, n), dtype=bf16)
    k32[:K] = rows
    return np.concatenate([k32] * 4, axis=0)


def _make_lhsT(q):
    x = np.ascontiguousarray(q.T).astype(np.float32)
    x2 = np.sum(q * q, axis=-1, dtype=np.float32)
    xh, xl = _split_bf16(x)
    x2h, x2l = _split_bf16(x2)
    ones = np.ones_like(x2, dtype=bf16)
    rows = np.concatenate([xh, xh, xl, x2h[None], x2l[None],
                           ones[None], ones[None]], axis=0)
    return _pad_bands(rows)


def _make_rhs(d):
    y = np.ascontiguousarray((-2.0 * d.T)).astype(np.float32)
    y2 = np.sum(d * d, axis=-1, dtype=np.float32)
    yh, yl = _split_bf16(y)
    y2h, y2l = _split_bf16(y2)
    ones = np.ones_like(y2, dtype=bf16)
    rows = np.concatenate([yh, yl, yh, ones[None], ones[None],
                           y2h[None], y2l[None]], axis=0)
    return _pad_bands(rows)


def make_in_maps(points1, points2):
    p1 = np.asarray(points1, dtype=np.float32)
    p2 = np.asarray(points2, dtype=np.float32)
    in_maps = []
    for i in range(N_CORES):
        b, h = divmod(i, 2)
        in_maps.append({
            "la": _make_lhsT(p1[b, h * NQ:(h + 1) * NQ]),
            "ra": _make_rhs(p2[b]),
        })
    return in_maps


def combine(results):
    """Host-side gather: dir1 sums + max-combine of negated dir2 partials."""
    total = 0.0
    for i in range(N_CORES):
        total += float(results[i]["out1"].astype(np.float64).sum())
    for b in range(B):
        r = np.maximum(results[2 * b]["out2"], results[2 * b + 1]["out2"])
        colmax = r.astype(np.float32).max(axis=0)
        total += float(np.maximum(-colmax, 0.0).astype(np.float64).sum())
    return np.float32(total / N)


_CACHE = {}


def kernel(points1, points2):
    from concourse.bass_utils import run_bass_kernel_spmd

    if "nc" not in _CACHE:
        _CACHE["nc"] = build_bass()
    nc = _CACHE["nc"]
    in_maps = make_in_maps(points1, points2)
    res = run_bass_kernel_spmd(nc, in_maps, core_ids=list(range(N_CORES)))
    return combine(res.results)
